# revision 21
# baseline (speedup 1.0000x reference)
"""Trainium2 Bass kernel for nn_NonOverlappingFlatVQVAE.

Strategy (8 NeuronCores, data-parallel over batch: 4 images/core):
 - Activations flow feature-major ([f, tokens]) so every layer's matmul
   contracts on partitions with host-pretransposed weights; the input
   patchify tiles are transposed on-chip via the PE transpose path.
 - VQ distance matrix uses the rank-256 factorization
   z.c_k = h3.(qc_w^T c_k) + qc_b.c_k, halving the contraction dim; the
   codebook projection cbP = qc_w^T @ codebook^T is precomputed on host in
   fp64.  argmin_k dist = argmax_k (h3.cbP_k - B_k) with
   B_k = |c_k|^2/2 - qc_b.c_k.  The dist matmul runs in bf16 (argmin
   margin on this data is >2.3; bf16 matmul noise is ~1e-3).
 - argmax per token: fused tensor_tensor_reduce (PSUM -> SBUF subtract +
   per-1024-chunk max accums), then a full-row max_index gives the first
   index attaining the global max (matches jnp.argmin tie-breaking).
 - Codebook rows are gathered straight from DRAM via indirect DMA.
 - Commitment loss via the identity mean((q-z)^2) =
   (sum z^2 - 2 sum_t gmax_t) / (N*D); only per-partition partial sums
   leave the device.
"""

import functools
import os
import sys

import numpy as np

sys.path.insert(0, "/opt/trn_rl_repo")

import concourse.bacc as bacc  # noqa: E402
import concourse.bass as bass  # noqa: E402
import concourse.mybir as mybir  # noqa: E402
from concourse.bass_utils import run_bass_kernel_spmd  # noqa: E402
from concourse.ap import AP  # noqa: E402
from concourse.masks import make_identity  # noqa: E402
from concourse.tile import TileContext  # noqa: E402

F32 = mybir.dt.float32
F16 = mybir.dt.float16
BF16 = mybir.dt.bfloat16
I32 = mybir.dt.int32
U32 = mybir.dt.uint32
AF = mybir.ActivationFunctionType
ALU = mybir.AluOpType
AX = mybir.AxisListType

P = 16          # patch size
B_FULL = 32     # full batch
N_CORES = 8
B_CORE = B_FULL // N_CORES   # images per core
CIN = 3
H = W = 256
HP = H // P                  # 16 patches per side
N_TOK = B_CORE * HP * HP     # 1024 tokens per core
N_TC = N_TOK // 128          # 8 token chunks
C = 256                      # mixer width
D = 512                      # code dim
K = 8192                     # codebook size
FIN = CIN * P * P            # 768 patchify features
NEG_INF = -3.0e38


def _build():
    stage = int(os.environ.get("KSTAGE", "9"))
    sub = int(os.environ.get("KSUB", "9"))
    nc = bacc.Bacc("TRN2", target_bir_lowering=False, debug=False,
                   num_devices=N_CORES)

    def dram(name, shape, dtype=F32, out=False):
        return nc.declare_dram_parameter(name, list(shape), dtype,
                                         isOutput=out)

    xs = dram("xs", [N_TOK, FIN])
    pe_wT = dram("pe_wT", [FIN, C])
    pe_b = dram("pe_b", [128, C // 128])
    mix1_wT = dram("mix1_wT", [C, C])
    mix1_b = dram("mix1_b", [128, C // 128])
    mix2_wT = dram("mix2_wT", [C, C])
    mix2_b = dram("mix2_b", [128, C // 128])
    qc_wT = dram("qc_wT", [C, D])
    qc_b = dram("qc_b", [128, D // 128])
    cbP = dram("cbP", [C, K], F16)
    negB = dram("negB", [1, K], F16)
    cbook = dram("cbook", [K, D])
    dpre1_wT = dram("dpre1_wT", [D, C])
    dpre1_b = dram("dpre1_b", [128, C // 128])
    dpre2_wT = dram("dpre2_wT", [C, C])
    dpre2_b = dram("dpre2_b", [128, C // 128])
    up_w2 = dram("up_w2", [C, FIN])
    up_b_bc = dram("up_b_bc", [128, FIN])

    dec_out = dram("dec_out", [N_TOK, FIN], out=True)
    idx_out = dram("idx_out", [128, N_TC], I32, out=True)
    loss_out = dram("loss_out", [128, 16], out=True)



    with TileContext(nc) as tc:
        with (
            tc.tile_pool(name="const", bufs=1) as cp,
            tc.tile_pool(name="stage", bufs=1) as sp,
            tc.tile_pool(name="work", bufs=3) as wp,
            tc.tile_pool(name="ps", bufs=2, space="PSUM") as pp,
            tc.tile_pool(name="psd", bufs=2, space="PSUM") as pd,
        ):
            ident = cp.tile([128, 128], F32)
            make_identity(nc, ident[:])

            def load_w(handle, kin, width):
                tiles = []
                for kc in range(kin):
                    t = cp.tile([128, width], F32, tag=f"w_{handle.name}{kc}", name=f"w_{handle.name}{kc}")
                    nc.sync.dma_start(out=t[:],
                                      in_=handle.ap()[kc * 128:(kc + 1) * 128, :])
                    tiles.append(t)
                return tiles

            peW = load_w(pe_wT, 6, C)
            m1W = load_w(mix1_wT, 2, C)
            m2W = load_w(mix2_wT, 2, C)
            qcW = load_w(qc_wT, 2, D)
            d1W = load_w(dpre1_wT, 4, C)
            d2W = load_w(dpre2_wT, 2, C)
            upW = load_w(up_w2, 2, FIN)

            def load_t(handle, shape, dtype=F32):
                t = cp.tile(list(shape), dtype, tag=f"c_{handle.name}", name=f"c_{handle.name}")
                nc.sync.dma_start(out=t[:], in_=handle.ap()[:])
                return t

            peB = load_t(pe_b, [128, 2])
            m1B = load_t(mix1_b, [128, 2])
            m2B = load_t(mix2_b, [128, 2])
            qcB = load_t(qc_b, [128, 4])
            d1B = load_t(dpre1_b, [128, 2])
            d2B = load_t(dpre2_b, [128, 2])
            upB = load_t(up_b_bc, [128, FIN])
            cbPb = []
            for kc in range(2):
                t = cp.tile([128, K], F16, tag=f"cbPb{kc}", name=f"cbPb{kc}")
                nc.sync.dma_start(out=t[:],
                                  in_=cbP.ap()[kc * 128:(kc + 1) * 128, :])
                cbPb.append(t)
            negBt = cp.tile([1, K], F16)
            nc.sync.dma_start(out=negBt[:], in_=negB.ap()[:])
            ones1 = cp.tile([1, 128], F16)
            nc.vector.memset(ones1[:], 1.0)

            neginf = cp.tile([128, 1], F32)
            nc.vector.memset(neginf[:], NEG_INF)
            zero8 = cp.tile([128, 8], F32)
            nc.vector.memset(zero8[:], 0.0)
            zsq = cp.tile([128, 8], F32)
            gcol = cp.tile([128, 8], F32)
            idxc = cp.tile([128, N_TC], I32)

            # ---------------- phase A: load + transpose input -------------
            xpool_ctx = tc.tile_pool(name="xin", bufs=1)
            xp = xpool_ctx.__enter__()
            xT = [xp.tile([128, N_TOK], F32, tag=f"xT{i}", name=f"xT{i}")
                  for i in range(6)]
            for b in range(B_CORE):
                for half in range(2):
                    tcn = b * 2 + half
                    x_sb = wp.tile([128, FIN], F32, tag="x_sb", name="x_sb")
                    nc.sync.dma_start(
                        out=x_sb[:],
                        in_=xs.ap()[tcn * 128:(tcn + 1) * 128, :])
                    for fc in range(6):
                        tp = pp.tile([128, 128], F32, tag="tps", name="tps")
                        nc.tensor.transpose(
                            out=tp[:], in_=x_sb[:, fc * 128:(fc + 1) * 128],
                            identity=ident[:])
                        nc.scalar.activation(
                            out=xT[fc][:, tcn * 128:(tcn + 1) * 128],
                            in_=tp[:], func=AF.Copy)

            # ---------------- phase B: encoder ----------------------------
            def linearT(src, w_tiles, bias, fout, relu, tagp, uniq=""):
                kin = len(w_tiles)
                mout = fout // 128
                dst = [sp.tile([128, N_TOK], F32, tag=f"{tagp}{m}",
                               name=f"{tagp}{m}{uniq}")
                       for m in range(mout)]
                for m in range(mout):
                    for n in range(N_TOK // 512):
                        ps = pp.tile([128, 512], F32, tag="enc_ps", name="enc_ps")
                        for kc in range(kin):
                            nc.tensor.matmul(
                                ps[:],
                                w_tiles[kc][:, m * 128:(m + 1) * 128],
                                src[kc][:, n * 512:(n + 1) * 512],
                                start=(kc == 0), stop=(kc == kin - 1))
                        nc.scalar.activation(
                            out=dst[m][:, n * 512:(n + 1) * 512], in_=ps[:],
                            func=AF.Relu if relu else AF.Identity,
                            bias=bias[:, m:m + 1], scale=1.0)
                return dst

            h1T = linearT(xT, peW, peB, C, True, "g1", uniq="h1")
            xpool_ctx.__exit__(None, None, None)
            h2T = linearT(h1T, m1W, m1B, C, True, "g2", uniq="h2")
            h3T = linearT(h2T, m2W, m2B, C, False, "g3", uniq="h3")

            h3b = []
            for m in range(2):
                t = cp.tile([128, N_TOK], F16, tag=f"h3b{m}", name=f"h3b{m}")
                nc.vector.tensor_copy(out=t[:], in_=h3T[m][:])
                h3b.append(t)

            # qc layer: only z^2 sums are needed (z itself never stored)
            for m in range(4):
                for n in range(2):
                    ps = pp.tile([128, 512], F32, tag="enc_ps", name="enc_ps")
                    for kc in range(2):
                        nc.tensor.matmul(
                            ps[:], qcW[kc][:, m * 128:(m + 1) * 128],
                            h3T[kc][:, n * 512:(n + 1) * 512],
                            start=(kc == 0), stop=(kc == 1))
                    zscr = wp.tile([128, 512], F32, tag="zscr", name="zscr")
                    nc.scalar.activation(
                        out=zscr[:], in_=ps[:], func=AF.Square,
                        bias=qcB[:, m:m + 1], scale=1.0,
                        accum_out=zsq[:, m * 2 + n:m * 2 + n + 1])

            # ---------------- phase C: VQ ---------------------------------
            vq_ctx = tc.tile_pool(name="vq", bufs=1)
            vp = vq_ctx.__enter__()
            qT = [cp.tile([128, N_TOK], F32, tag=f"qT{m}", name=f"qT{m}")
                  for m in range(4)]
            if stage < 2:
                nc.vector.memset(idxc[:], 0)
                nc.vector.memset(gcol[:], 0.0)
            for tcn in range(N_TC if stage >= 2 else 0):
                ts = tcn * 128
                ndq = vp.tile([128, K], F32, tag="ndq", name="ndq")
                cmax = wp.tile([128, 8], F32, tag="cmax", name="cmax")
                for kb in range(8):
                    ps = pd.tile([128, 1024], F32, tag="dist_ps", name="dist_ps")
                    for n in range(2):
                        sl = slice(kb * 1024 + n * 512,
                                   kb * 1024 + (n + 1) * 512)
                        for c in range(2):
                            nc.tensor.matmul(
                                ps[:, n * 512:(n + 1) * 512],
                                h3b[c][:, ts:ts + 128],
                                cbPb[c][:, sl],
                                start=(c == 0), stop=False)
                        nc.tensor.matmul(
                            ps[:, n * 512:(n + 1) * 512],
                            ones1[:], negBt[:, sl],
                            start=False, stop=True)
                    nc.vector.tensor_scalar(
                        out=ndq[:, kb * 1024:(kb + 1) * 1024],
                        in0=ps[:], scalar1=0.0, scalar2=None,
                        op0=ALU.add, op1=ALU.max,
                        accum_out=cmax[:, kb:kb + 1])
                gm = wp.tile([128, 1], F32, tag="gm", name="gm")
                nc.vector.tensor_reduce(out=gm[:], in_=cmax[:], axis=AX.X,
                                        op=ALU.max)
                nc.vector.tensor_copy(out=gcol[:, tcn:tcn + 1], in_=gm[:])
                if sub < 2:
                    nc.vector.memset(idxc[:, tcn:tcn + 1], 0)
                    continue
                gm8 = wp.tile([128, 8], F32, tag="gm8", name="gm8")
                nc.vector.tensor_scalar(out=gm8[:], in0=zero8[:],
                                        scalar1=gm[:, 0:1], scalar2=None,
                                        op0=ALU.add)
                mi = wp.tile([128, 8], U32, tag="mi", name="mi")
                nc.vector.max_index(out=mi[:], in_max=gm8[:], in_values=ndq[:])
                idx32 = wp.tile([128, 1], I32, tag="idx32", name="idx32")
                nc.vector.tensor_copy(out=idx32[:], in_=mi[:, 0:1])
                nc.vector.tensor_copy(out=idxc[:, tcn:tcn + 1], in_=idx32[:])

                if stage < 3:
                    continue
                q_sb = wp.tile([128, D], F32, tag="q_sb", name="q_sb")
                nc.gpsimd.indirect_dma_start(
                    out=q_sb[:], out_offset=None, in_=cbook.ap()[:],
                    in_offset=bass.IndirectOffsetOnAxis(ap=idx32[:, 0:1],
                                                        axis=0))
                for m in range(4):
                    tp = pp.tile([128, 128], F32, tag="tps", name="tps")
                    nc.tensor.transpose(
                        out=tp[:], in_=q_sb[:, m * 128:(m + 1) * 128],
                        identity=ident[:])
                    nc.scalar.activation(out=qT[m][:, ts:ts + 128],
                                         in_=tp[:], func=AF.Copy)

            nc.sync.dma_start(out=idx_out.ap()[:], in_=idxc[:])
            nc.sync.dma_start(out=loss_out.ap()[:, 0:8], in_=zsq[:])
            nc.sync.dma_start(out=loss_out.ap()[:, 8:16], in_=gcol[:])

            vq_ctx.__exit__(None, None, None)

            # ---------------- phase E: decoder ----------------------------
            if stage >= 4:
                e1T = linearT(qT, d1W, d1B, C, True, "g1", uniq="e1")
                e2T = linearT(e1T, d2W, d2B, C, True, "g2", uniq="e2")
            for b in range(B_CORE):
                for half in range(2):
                    tcn = b * 2 + half
                    ts = tcn * 128
                    dec_sb = wp.tile([128, FIN], F32, tag="dec_sb", name="dec_sb")
                    if stage >= 4:
                        ps = pd.tile([128, FIN], F32, tag="dist_ps", name="dec_ps")
                        for lo, hi in ((0, 512), (512, 768)):
                            for c in range(2):
                                nc.tensor.matmul(
                                    ps[:, lo:hi],
                                    e2T[c][:, ts:ts + 128],
                                    upW[c][:, lo:hi],
                                    start=(c == 0), stop=(c == 1))
                        nc.vector.tensor_tensor(out=dec_sb[:], in0=ps[:],
                                                in1=upB[:], op=ALU.add)
                    else:
                        nc.vector.memset(dec_sb[:], 0.0)
                    nc.sync.dma_start(
                        out=dec_out.ap()[tcn * 128:(tcn + 1) * 128, :],
                        in_=dec_sb[:])

    nc.compile()
    return nc


@functools.lru_cache(maxsize=4)
def _get_nc():
    return _build()


def _host_prep(inputs):
    """Precompute host-side tensors shared by all cores (fp64 where it
    matters for the VQ argmin)."""
    f64 = {k: np.asarray(v, np.float64) for k, v in inputs.items()}
    cb = f64["codebook"]
    cbP = (f64["qc_w"].T @ cb.T)                       # [C, K]
    qb_c = f64["qc_b"] @ cb.T                          # [K]
    Bvec = 0.5 * (cb * cb).sum(-1) - qb_c              # [K]

    def colsplit(v, nm):
        # bias vector [f] -> [128, f//128] with tile m in column m
        return np.ascontiguousarray(
            np.asarray(v, np.float32).reshape(nm, 128).T)

    common = {
        "pe_wT": np.ascontiguousarray(
            np.asarray(inputs["pe_w"], np.float32)
            .reshape(C, FIN).T),
        "pe_b": colsplit(inputs["pe_b"], 2),
        "mix1_wT": np.ascontiguousarray(np.asarray(inputs["mix1_w"],
                                                   np.float32).T),
        "mix1_b": colsplit(inputs["mix1_b"], 2),
        "mix2_wT": np.ascontiguousarray(np.asarray(inputs["mix2_w"],
                                                   np.float32).T),
        "mix2_b": colsplit(inputs["mix2_b"], 2),
        "qc_wT": np.ascontiguousarray(np.asarray(inputs["qc_w"],
                                                 np.float32).T),
        "qc_b": colsplit(inputs["qc_b"], 4),
        "cbP": cbP.astype(np.float16),
        "negB": (-Bvec).astype(np.float16)[None, :],
        "cbook": np.asarray(inputs["codebook"], np.float32),
        "dpre1_wT": np.ascontiguousarray(np.asarray(inputs["dpre1_w"],
                                                    np.float32).T),
        "dpre1_b": colsplit(inputs["dpre1_b"], 2),
        "dpre2_wT": np.ascontiguousarray(np.asarray(inputs["dpre2_w"],
                                                    np.float32).T),
        "dpre2_b": colsplit(inputs["dpre2_b"], 2),
        "up_w2": np.ascontiguousarray(
            np.asarray(inputs["up_w"], np.float32).reshape(C, FIN)),
        "up_b_bc": np.tile(
            np.repeat(np.asarray(inputs["up_b"], np.float32), P * P)[None, :],
            (128, 1)),
    }
    return common


def kernel(**inputs):
    nc = _get_nc()
    common = _host_prep(inputs)
    x = np.asarray(inputs["x"], np.float32)

    in_maps = []
    for core in range(N_CORES):
        m = dict(common)
        xsh = x[core * B_CORE:(core + 1) * B_CORE]
        xtok = xsh.reshape(B_CORE, CIN, HP, P, HP, P).transpose(
            0, 2, 4, 1, 3, 5).reshape(N_TOK, FIN)
        m["xs"] = np.ascontiguousarray(xtok)
        in_maps.append(m)

    res = run_bass_kernel_spmd(nc, in_maps, list(range(N_CORES)))

    dec = np.empty((B_FULL, CIN, H, W), np.float32)
    idx = np.empty((B_FULL, HP * HP), np.int32)
    loss_num = 0.0
    for core in range(N_CORES):
        r = res.results[core]
        dec[core * B_CORE:(core + 1) * B_CORE] = (
            r["dec_out"].reshape(B_CORE, HP, HP, CIN, P, P)
            .transpose(0, 3, 1, 4, 2, 5).reshape(B_CORE, CIN, H, W))
        # idx_out[p, tcn] is token tcn*128+p; tokens are (b, i, j) flat
        idx_core = np.ascontiguousarray(r["idx_out"].T).reshape(B_CORE,
                                                                HP * HP)
        idx[core * B_CORE:(core + 1) * B_CORE] = idx_core
        lo = r["loss_out"].astype(np.float64)
        loss_num += lo[:, 0:8].sum() - 2.0 * lo[:, 8:16].sum()
    diff = np.float32(loss_num / (B_FULL * HP * HP * D))
    return dec, np.asarray([diff], np.float32), idx


# revision 22
# speedup vs baseline: 1.0150x; 1.0150x over previous
"""Trainium2 Bass kernel for nn_NonOverlappingFlatVQVAE.

Strategy (8 NeuronCores, data-parallel over batch: 4 images/core):
 - Activations flow feature-major ([f, tokens]) so every layer's matmul
   contracts on partitions with host-pretransposed weights; the input
   patchify tiles are transposed on-chip via the PE transpose path.
 - VQ distance matrix uses the rank-256 factorization
   z.c_k = h3.(qc_w^T c_k) + qc_b.c_k, halving the contraction dim; the
   codebook projection cbP = qc_w^T @ codebook^T is precomputed on host in
   fp64.  argmin_k dist = argmax_k (h3.cbP_k - B_k) with
   B_k = |c_k|^2/2 - qc_b.c_k.  The dist matmul runs in bf16 (argmin
   margin on this data is >2.3; bf16 matmul noise is ~1e-3).
 - argmax per token: fused tensor_tensor_reduce (PSUM -> SBUF subtract +
   per-1024-chunk max accums), then a full-row max_index gives the first
   index attaining the global max (matches jnp.argmin tie-breaking).
 - Codebook rows are gathered straight from DRAM via indirect DMA.
 - Commitment loss via the identity mean((q-z)^2) =
   (sum z^2 - 2 sum_t gmax_t) / (N*D); only per-partition partial sums
   leave the device.
"""

import functools
import os
import sys

import numpy as np

sys.path.insert(0, "/opt/trn_rl_repo")

import concourse.bacc as bacc  # noqa: E402
import concourse.bass as bass  # noqa: E402
import concourse.mybir as mybir  # noqa: E402
from concourse.bass_utils import run_bass_kernel_spmd  # noqa: E402
from concourse.masks import make_identity  # noqa: E402
from concourse.tile import TileContext  # noqa: E402

F32 = mybir.dt.float32
F16 = mybir.dt.float16
BF16 = mybir.dt.bfloat16
I32 = mybir.dt.int32
U32 = mybir.dt.uint32
AF = mybir.ActivationFunctionType
ALU = mybir.AluOpType
AX = mybir.AxisListType

P = 16          # patch size
B_FULL = 32     # full batch
N_CORES = 8
B_CORE = B_FULL // N_CORES   # images per core
CIN = 3
H = W = 256
HP = H // P                  # 16 patches per side
N_TOK = B_CORE * HP * HP     # 1024 tokens per core
N_TC = N_TOK // 128          # 8 token chunks
C = 256                      # mixer width
D = 512                      # code dim
K = 8192                     # codebook size
FIN = CIN * P * P            # 768 patchify features
NEG_INF = -3.0e38


def _build():
    stage = int(os.environ.get("KSTAGE", "9"))
    sub = int(os.environ.get("KSUB", "9"))
    nc = bacc.Bacc("TRN2", target_bir_lowering=False, debug=False,
                   num_devices=N_CORES)

    def dram(name, shape, dtype=F32, out=False):
        return nc.declare_dram_parameter(name, list(shape), dtype,
                                         isOutput=out)

    xs = dram("xs", [N_TOK, FIN])
    pe_wT = dram("pe_wT", [FIN, C])
    pe_b = dram("pe_b", [128, C // 128])
    mix1_wT = dram("mix1_wT", [C, C])
    mix1_b = dram("mix1_b", [128, C // 128])
    mix2_wT = dram("mix2_wT", [C, C])
    mix2_b = dram("mix2_b", [128, C // 128])
    qc_wT = dram("qc_wT", [C, D])
    qc_b = dram("qc_b", [128, D // 128])
    cbP = dram("cbP", [C, K], F16)
    negB = dram("negB", [1, K], F16)
    cbook = dram("cbook", [K, D])
    dpre1_wT = dram("dpre1_wT", [D, C])
    dpre1_b = dram("dpre1_b", [128, C // 128])
    dpre2_wT = dram("dpre2_wT", [C, C])
    dpre2_b = dram("dpre2_b", [128, C // 128])
    up_w2 = dram("up_w2", [C, FIN])
    up_b_bc = dram("up_b_bc", [128, FIN])

    dec_out = dram("dec_out", [N_TOK, FIN], out=True)
    idx_out = dram("idx_out", [128, N_TC], I32, out=True)
    loss_out = dram("loss_out", [128, 16], out=True)



    with TileContext(nc) as tc:
        with (
            tc.tile_pool(name="const", bufs=1) as cp,
            tc.tile_pool(name="stage", bufs=1) as sp,
            tc.tile_pool(name="work", bufs=3) as wp,
            tc.tile_pool(name="ps", bufs=2, space="PSUM") as pp,
            tc.tile_pool(name="psd", bufs=2, space="PSUM") as pd,
        ):
            ident = cp.tile([128, 128], F32)
            make_identity(nc, ident[:])

            def load_w(handle, kin, width):
                tiles = []
                for kc in range(kin):
                    t = cp.tile([128, width], F32, tag=f"w_{handle.name}{kc}", name=f"w_{handle.name}{kc}")
                    nc.sync.dma_start(out=t[:],
                                      in_=handle.ap()[kc * 128:(kc + 1) * 128, :])
                    tiles.append(t)
                return tiles

            peW = load_w(pe_wT, 6, C)
            m1W = load_w(mix1_wT, 2, C)
            m2W = load_w(mix2_wT, 2, C)
            qcW = load_w(qc_wT, 2, D)
            d1W = load_w(dpre1_wT, 4, C)
            d2W = load_w(dpre2_wT, 2, C)
            upW = load_w(up_w2, 2, FIN)

            def load_t(handle, shape, dtype=F32):
                t = cp.tile(list(shape), dtype, tag=f"c_{handle.name}", name=f"c_{handle.name}")
                nc.sync.dma_start(out=t[:], in_=handle.ap()[:])
                return t

            peB = load_t(pe_b, [128, 2])
            m1B = load_t(mix1_b, [128, 2])
            m2B = load_t(mix2_b, [128, 2])
            qcB = load_t(qc_b, [128, 4])
            d1B = load_t(dpre1_b, [128, 2])
            d2B = load_t(dpre2_b, [128, 2])
            upB = load_t(up_b_bc, [128, FIN])
            cbPb = []
            for kc in range(2):
                t = cp.tile([128, K], F16, tag=f"cbPb{kc}", name=f"cbPb{kc}")
                nc.sync.dma_start(out=t[:],
                                  in_=cbP.ap()[kc * 128:(kc + 1) * 128, :])
                cbPb.append(t)
            negBt = cp.tile([1, K], F16)
            nc.sync.dma_start(out=negBt[:], in_=negB.ap()[:])
            ones1 = cp.tile([1, 128], F16)
            nc.vector.memset(ones1[:], 1.0)

            neginf = cp.tile([128, 1], F32)
            nc.vector.memset(neginf[:], NEG_INF)
            zero8 = cp.tile([128, 8], F32)
            nc.vector.memset(zero8[:], 0.0)
            zsq = cp.tile([128, 8], F32)
            gcol = cp.tile([128, 8], F32)
            idxc = cp.tile([128, N_TC], I32)

            # ---------------- phase A: load + transpose input -------------
            xpool_ctx = tc.tile_pool(name="xin", bufs=1)
            xp = xpool_ctx.__enter__()
            xT = [xp.tile([128, N_TOK], F32, tag=f"xT{i}", name=f"xT{i}")
                  for i in range(6)]
            for b in range(B_CORE):
                for half in range(2):
                    tcn = b * 2 + half
                    x_sb = wp.tile([128, FIN], F32, tag="x_sb", name="x_sb")
                    nc.sync.dma_start(
                        out=x_sb[:],
                        in_=xs.ap()[tcn * 128:(tcn + 1) * 128, :])
                    for fc in range(6):
                        tp = pp.tile([128, 128], F32, tag="tps", name="tps")
                        nc.tensor.transpose(
                            out=tp[:], in_=x_sb[:, fc * 128:(fc + 1) * 128],
                            identity=ident[:])
                        nc.scalar.activation(
                            out=xT[fc][:, tcn * 128:(tcn + 1) * 128],
                            in_=tp[:], func=AF.Copy)

            # ---------------- phase B: encoder ----------------------------
            def linearT(src, w_tiles, bias, fout, relu, tagp, uniq=""):
                kin = len(w_tiles)
                mout = fout // 128
                dst = [sp.tile([128, N_TOK], F32, tag=f"{tagp}{m}",
                               name=f"{tagp}{m}{uniq}")
                       for m in range(mout)]
                for m in range(mout):
                    for n in range(N_TOK // 512):
                        ps = pp.tile([128, 512], F32, tag="enc_ps", name="enc_ps")
                        for kc in range(kin):
                            nc.tensor.matmul(
                                ps[:],
                                w_tiles[kc][:, m * 128:(m + 1) * 128],
                                src[kc][:, n * 512:(n + 1) * 512],
                                start=(kc == 0), stop=(kc == kin - 1))
                        nc.scalar.activation(
                            out=dst[m][:, n * 512:(n + 1) * 512], in_=ps[:],
                            func=AF.Relu if relu else AF.Identity,
                            bias=bias[:, m:m + 1], scale=1.0)
                return dst

            h1T = linearT(xT, peW, peB, C, True, "g1", uniq="h1")
            xpool_ctx.__exit__(None, None, None)
            h2T = linearT(h1T, m1W, m1B, C, True, "g2", uniq="h2")
            h3T = linearT(h2T, m2W, m2B, C, False, "g3", uniq="h3")

            h3b = []
            for m in range(2):
                t = cp.tile([128, N_TOK], F16, tag=f"h3b{m}", name=f"h3b{m}")
                nc.vector.tensor_copy(out=t[:], in_=h3T[m][:])
                h3b.append(t)

            # qc layer: only z^2 sums are needed (z itself never stored)
            for m in range(4):
                for n in range(2):
                    ps = pp.tile([128, 512], F32, tag="enc_ps", name="enc_ps")
                    for kc in range(2):
                        nc.tensor.matmul(
                            ps[:], qcW[kc][:, m * 128:(m + 1) * 128],
                            h3T[kc][:, n * 512:(n + 1) * 512],
                            start=(kc == 0), stop=(kc == 1))
                    zscr = wp.tile([128, 512], F32, tag="zscr", name="zscr")
                    nc.scalar.activation(
                        out=zscr[:], in_=ps[:], func=AF.Square,
                        bias=qcB[:, m:m + 1], scale=1.0,
                        accum_out=zsq[:, m * 2 + n:m * 2 + n + 1])

            # ---------------- phase C: VQ ---------------------------------
            vq_ctx = tc.tile_pool(name="vq", bufs=1)
            vp = vq_ctx.__enter__()
            qT = [cp.tile([128, N_TOK], F32, tag=f"qT{m}", name=f"qT{m}")
                  for m in range(4)]
            if stage < 2:
                nc.vector.memset(idxc[:], 0)
                nc.vector.memset(gcol[:], 0.0)
            for tcn in range(N_TC if stage >= 2 else 0):
                ts = tcn * 128
                ndq = vp.tile([128, K], F32, tag="ndq", name="ndq")
                cmax = wp.tile([128, 8], F32, tag="cmax", name="cmax")
                for kb in range(8):
                    ps = pd.tile([128, 1024], F32, tag="dist_ps", name="dist_ps")
                    for n in range(2):
                        sl = slice(kb * 1024 + n * 512,
                                   kb * 1024 + (n + 1) * 512)
                        for c in range(2):
                            nc.tensor.matmul(
                                ps[:, n * 512:(n + 1) * 512],
                                h3b[c][:, ts:ts + 128],
                                cbPb[c][:, sl],
                                start=(c == 0), stop=False)
                        nc.tensor.matmul(
                            ps[:, n * 512:(n + 1) * 512],
                            ones1[:], negBt[:, sl],
                            start=False, stop=True)
                    nc.vector.tensor_scalar(
                        out=ndq[:, kb * 1024:(kb + 1) * 1024],
                        in0=ps[:], scalar1=0.0, scalar2=None,
                        op0=ALU.add, op1=ALU.max,
                        accum_out=cmax[:, kb:kb + 1])
                gm = wp.tile([128, 1], F32, tag="gm", name="gm")
                nc.vector.tensor_reduce(out=gm[:], in_=cmax[:], axis=AX.X,
                                        op=ALU.max)
                nc.vector.tensor_copy(out=gcol[:, tcn:tcn + 1], in_=gm[:])
                if sub < 2:
                    nc.vector.memset(idxc[:, tcn:tcn + 1], 0)
                    continue
                gm8 = wp.tile([128, 8], F32, tag="gm8", name="gm8")
                nc.vector.tensor_scalar(out=gm8[:], in0=zero8[:],
                                        scalar1=gm[:, 0:1], scalar2=None,
                                        op0=ALU.add)
                mi = wp.tile([128, 8], U32, tag="mi", name="mi")
                nc.vector.max_index(out=mi[:], in_max=gm8[:], in_values=ndq[:])
                idx32 = wp.tile([128, 1], I32, tag="idx32", name="idx32")
                nc.vector.tensor_copy(out=idx32[:], in_=mi[:, 0:1])
                nc.vector.tensor_copy(out=idxc[:, tcn:tcn + 1], in_=idx32[:])

                if stage < 3:
                    continue
                q_sb = wp.tile([128, D], F32, tag="q_sb", name="q_sb")
                nc.gpsimd.indirect_dma_start(
                    out=q_sb[:], out_offset=None, in_=cbook.ap()[:],
                    in_offset=bass.IndirectOffsetOnAxis(ap=idx32[:, 0:1],
                                                        axis=0))
                for m in range(4):
                    tp = pp.tile([128, 128], F32, tag="tps", name="tps")
                    nc.tensor.transpose(
                        out=tp[:], in_=q_sb[:, m * 128:(m + 1) * 128],
                        identity=ident[:])
                    nc.scalar.activation(out=qT[m][:, ts:ts + 128],
                                         in_=tp[:], func=AF.Copy)

            nc.sync.dma_start(out=idx_out.ap()[:], in_=idxc[:])
            nc.sync.dma_start(out=loss_out.ap()[:, 0:8], in_=zsq[:])
            nc.sync.dma_start(out=loss_out.ap()[:, 8:16], in_=gcol[:])

            vq_ctx.__exit__(None, None, None)

            # ---------------- phase E: decoder ----------------------------
            if stage >= 4:
                e1T = linearT(qT, d1W, d1B, C, True, "g1", uniq="e1")
                e2T = linearT(e1T, d2W, d2B, C, True, "g2", uniq="e2")
            for b in range(B_CORE):
                for half in range(2):
                    tcn = b * 2 + half
                    ts = tcn * 128
                    dec_sb = wp.tile([128, FIN], F32, tag="dec_sb", name="dec_sb")
                    if stage >= 4:
                        ps = pd.tile([128, FIN], F32, tag="dist_ps", name="dec_ps")
                        for lo, hi in ((0, 512), (512, 768)):
                            for c in range(2):
                                nc.tensor.matmul(
                                    ps[:, lo:hi],
                                    e2T[c][:, ts:ts + 128],
                                    upW[c][:, lo:hi],
                                    start=(c == 0), stop=(c == 1))
                        nc.vector.tensor_tensor(out=dec_sb[:], in0=ps[:],
                                                in1=upB[:], op=ALU.add)
                    else:
                        nc.vector.memset(dec_sb[:], 0.0)
                    nc.sync.dma_start(
                        out=dec_out.ap()[tcn * 128:(tcn + 1) * 128, :],
                        in_=dec_sb[:])

    nc.compile()
    return nc


@functools.lru_cache(maxsize=4)
def _get_nc():
    return _build()


def _host_prep(inputs):
    """Precompute host-side tensors shared by all cores (fp64 where it
    matters for the VQ argmin)."""
    f64 = {k: np.asarray(v, np.float64) for k, v in inputs.items()}
    cb = f64["codebook"]
    cbP = (f64["qc_w"].T @ cb.T)                       # [C, K]
    qb_c = f64["qc_b"] @ cb.T                          # [K]
    Bvec = 0.5 * (cb * cb).sum(-1) - qb_c              # [K]

    def colsplit(v, nm):
        # bias vector [f] -> [128, f//128] with tile m in column m
        return np.ascontiguousarray(
            np.asarray(v, np.float32).reshape(nm, 128).T)

    common = {
        "pe_wT": np.ascontiguousarray(
            np.asarray(inputs["pe_w"], np.float32)
            .reshape(C, FIN).T),
        "pe_b": colsplit(inputs["pe_b"], 2),
        "mix1_wT": np.ascontiguousarray(np.asarray(inputs["mix1_w"],
                                                   np.float32).T),
        "mix1_b": colsplit(inputs["mix1_b"], 2),
        "mix2_wT": np.ascontiguousarray(np.asarray(inputs["mix2_w"],
                                                   np.float32).T),
        "mix2_b": colsplit(inputs["mix2_b"], 2),
        "qc_wT": np.ascontiguousarray(np.asarray(inputs["qc_w"],
                                                 np.float32).T),
        "qc_b": colsplit(inputs["qc_b"], 4),
        "cbP": cbP.astype(np.float16),
        "negB": (-Bvec).astype(np.float16)[None, :],
        "cbook": np.asarray(inputs["codebook"], np.float32),
        "dpre1_wT": np.ascontiguousarray(np.asarray(inputs["dpre1_w"],
                                                    np.float32).T),
        "dpre1_b": colsplit(inputs["dpre1_b"], 2),
        "dpre2_wT": np.ascontiguousarray(np.asarray(inputs["dpre2_w"],
                                                    np.float32).T),
        "dpre2_b": colsplit(inputs["dpre2_b"], 2),
        "up_w2": np.ascontiguousarray(
            np.asarray(inputs["up_w"], np.float32).reshape(C, FIN)),
        "up_b_bc": np.tile(
            np.repeat(np.asarray(inputs["up_b"], np.float32), P * P)[None, :],
            (128, 1)),
    }
    return common


def kernel(**inputs):
    nc = _get_nc()
    common = _host_prep(inputs)
    x = np.asarray(inputs["x"], np.float32)

    in_maps = []
    for core in range(N_CORES):
        m = dict(common)
        xsh = x[core * B_CORE:(core + 1) * B_CORE]
        xtok = xsh.reshape(B_CORE, CIN, HP, P, HP, P).transpose(
            0, 2, 4, 1, 3, 5).reshape(N_TOK, FIN)
        m["xs"] = np.ascontiguousarray(xtok)
        in_maps.append(m)

    res = run_bass_kernel_spmd(nc, in_maps, list(range(N_CORES)))

    dec = np.empty((B_FULL, CIN, H, W), np.float32)
    idx = np.empty((B_FULL, HP * HP), np.int32)
    loss_num = 0.0
    for core in range(N_CORES):
        r = res.results[core]
        dec[core * B_CORE:(core + 1) * B_CORE] = (
            r["dec_out"].reshape(B_CORE, HP, HP, CIN, P, P)
            .transpose(0, 3, 1, 4, 2, 5).reshape(B_CORE, CIN, H, W))
        # idx_out[p, tcn] is token tcn*128+p; tokens are (b, i, j) flat
        idx_core = np.ascontiguousarray(r["idx_out"].T).reshape(B_CORE,
                                                                HP * HP)
        idx[core * B_CORE:(core + 1) * B_CORE] = idx_core
        lo = r["loss_out"].astype(np.float64)
        loss_num += lo[:, 0:8].sum() - 2.0 * lo[:, 8:16].sum()
    diff = np.float32(loss_num / (B_FULL * HP * HP * D))
    return dec, np.asarray([diff], np.float32), idx
